# revision 1
# baseline (speedup 1.0000x reference)
"""Causal GQA self-attention (B=4, T=2048, C=2048, 16 Q heads / 8 KV heads,
hd=128) as a Bass/Tile SPMD kernel on 8 Trainium2 NeuronCores.

Sharding: core c = (batch b = c//2, head-group g = c%2). Each core handles one
batch and 8 Q heads / 4 KV heads. Wq/Wk/Wv column-sharded on the head dim, Wo
row-sharded; the host sums the two partial Wo products per batch (2-way
all-reduce done on host during the gather).

All on-device tensors live in a transposed [feature, token] layout so every
matmul contraction sits on the partition dim with no on-device transposes:
  qT/kT = [d, t], v = [t, d], scores as S^T = [k, q], output as y^T = [o, t].
Bulk matmuls run in bf16 (fp32 PSUM accumulation; ~4e-3 end-to-end rel err).
The loop is software-pipelined: attention/Wo of block tb-1 interleave with
the projections of block tb so projection matmuls fill PE gaps while the
ScalarE exp stream drains; softmax denominators accumulate on the PE via an
accumulating ones-matmul, reciprocals use the single-op approx DVE path, and
causal masking is a GpSimd memset + one [128,128] triangular multiply.
"""

import sys

import ml_dtypes
import numpy as np

sys.path.insert(0, "/opt/trn_rl_repo")

import concourse.bass as bass  # noqa: E402
import concourse.mybir as mybir  # noqa: E402
import concourse.tile as tile  # noqa: E402
from concourse import bacc  # noqa: E402
from concourse.bass_utils import run_bass_kernel_spmd  # noqa: E402

# Problem shape (hardcoded per contest contract).
B = 4
T = 2048
C = 2048
HD = 128
N_HEAD = 16
N_KV_HEAD = 8
NQH = N_HEAD // 2  # q heads per core (group)
NKV = N_KV_HEAD // 2  # kv heads per core
TB = 512  # token block
NTB = T // TB
NCT = C // 128  # contraction tiles for the projections
SCALE = 1.0 / float(np.sqrt(HD))

F32 = mybir.dt.float32
F32R = mybir.dt.float32r
BF16 = mybir.dt.bfloat16
MULT = mybir.AluOpType.mult
ADD = mybir.AluOpType.add
EXP = mybir.ActivationFunctionType.Exp


def _rope(nc, tmpp, dst, src_psum, cosb, nsinb):
    """dst = src*cos + rot_half(src)*sin, src in [d, t] layout (d partitions).

    rot_half(x)[d] = -x[d+64] for d<64, +x[d-64] for d>=64; the sign lives in
    nsinb so both halves are plain multiplies. nsinb is the sin table rotated
    by 64 partitions (nsinb[64+i] = -sin[i], nsinb[i] = sin[64+i]) so each
    tensor_tensor has equal base partitions on its two SBUF inputs (HW rule).
    """
    t0 = tmpp.tile([HD, TB], F32, tag="t0")
    nc.scalar.copy(t0[:], src_psum[:])
    nc.vector.tensor_mul(dst, t0[:], cosb[:])
    t2 = tmpp.tile([HD, TB], F32, tag="t2")
    nc.vector.tensor_mul(t2[0:64, :], t0[64:128, :], nsinb[64:128, :])
    nc.vector.tensor_mul(t2[64:128, :], t0[0:64, :], nsinb[0:64, :])
    nc.vector.scalar_tensor_tensor(dst, t2[:], 1.0, dst, op0=MULT, op1=ADD)


def build_nc():
    nc = bacc.Bacc("TRN2", target_bir_lowering=False, debug=False, num_devices=8)

    xT = nc.dram_tensor("xT", [C, T], BF16, kind="ExternalInput")
    wqT = nc.dram_tensor("wqT", [C, NQH * HD], BF16, kind="ExternalInput")
    wkT = nc.dram_tensor("wkT", [C, NKV * HD], BF16, kind="ExternalInput")
    wvT = nc.dram_tensor("wvT", [C, NKV * HD], BF16, kind="ExternalInput")
    woT = nc.dram_tensor("woT", [NQH * HD, C], BF16, kind="ExternalInput")
    cosdt = nc.dram_tensor("cosdt", [HD, T], F32, kind="ExternalInput")
    nsindt = nc.dram_tensor("nsindt", [HD, T], F32, kind="ExternalInput")
    masks = nc.dram_tensor("masks", [4, 128, TB], BF16, kind="ExternalInput")
    onescol = nc.dram_tensor("onescol", [128, 1], BF16, kind="ExternalInput")
    onesrow = nc.dram_tensor("onesrow", [1, 128], F32R, kind="ExternalInput")
    yT = nc.dram_tensor("yT", [C, T], F32, kind="ExternalOutput")

    from contextlib import ExitStack

    with ExitStack() as es:
        tc = es.enter_context(tile.TileContext(nc))
        es.enter_context(nc.allow_low_precision("fp32r attention"))
        constp = es.enter_context(tc.tile_pool(name="const", bufs=1))
        strp = es.enter_context(tc.tile_pool(name="stream", bufs=2))
        perp = es.enter_context(tc.tile_pool(name="persist", bufs=1))
        xp = es.enter_context(tc.tile_pool(name="xp", bufs=16))
        wqp = es.enter_context(tc.tile_pool(name="wq", bufs=2))
        wkp = es.enter_context(tc.tile_pool(name="wk", bufs=2))
        wvp = es.enter_context(tc.tile_pool(name="wv", bufs=2))
        wop = es.enter_context(tc.tile_pool(name="wo", bufs=3))
        qp = es.enter_context(tc.tile_pool(name="qt", bufs=16))
        outp = es.enter_context(tc.tile_pool(name="ot", bufs=8))
        tmpp = es.enter_context(tc.tile_pool(name="tmp", bufs=2))
        expp = es.enter_context(tc.tile_pool(name="exps", bufs=8))
        denp = es.enter_context(tc.tile_pool(name="den", bufs=2))
        smallp = es.enter_context(tc.tile_pool(name="small", bufs=2))
        yp = es.enter_context(tc.tile_pool(name="ysb", bufs=2))
        projp = es.enter_context(tc.tile_pool(name="pp", bufs=3, space="PSUM"))
        spsum = es.enter_context(tc.tile_pool(name="sp", bufs=3, space="PSUM"))
        opsum = es.enter_context(tc.tile_pool(name="op", bufs=2, space="PSUM"))
        if True:
            mask_t = []
            for m in range(4):
                mt = constp.tile([128, TB], BF16, tag=f"mask{m}")
                nc.sync.dma_start(mt[:], masks[m])
                mask_t.append(mt)
            ones_c = constp.tile([128, 1], BF16, tag="onesc")
            nc.sync.dma_start(ones_c[:], onescol[:])
            ones_r = constp.tile([1, 128], F32R, tag="onesr")
            nc.sync.dma_start(ones_r[:], onesrow[:])

            kT = [perp.tile([HD, T], BF16, tag=f"kT{h}", name=f"kT{h}") for h in range(NKV)]
            vT = [perp.tile([128, NKV * HD], BF16, tag=f"v{i}", name=f"v{i}") for i in range(T // 128)]

            def load_block(tb):
                tsl = slice(tb * TB, (tb + 1) * TB)
                xb = []
                for ct in range(NCT):
                    t_ = xp.tile([128, TB], BF16, tag="xb", name=f"xb{tb}_{ct}")
                    nc.sync.dma_start(t_[:], xT[ct * 128 : (ct + 1) * 128, tsl])
                    xb.append(t_)
                cosb = strp.tile([HD, TB], F32, tag="cosb", name=f"cosb{tb}")
                nc.sync.dma_start(cosb[:], cosdt[:, tsl])
                nsinb = strp.tile([HD, TB], F32, tag="nsinb", name=f"nsinb{tb}")
                nc.sync.dma_start(nsinb[:], nsindt[:, tsl])
                return xb, cosb, nsinb

            def proj_block(tb, xb, cosb, nsinb):
                tsl = slice(tb * TB, (tb + 1) * TB)
                # K projection (k^T layout [d, t]) + RoPE
                for kw in range(2):
                    kps = [projp.tile([128, TB], F32, tag="pp", name=f"kps{tb}_{kw}_{i}") for i in range(2)]
                    for ct in range(NCT):
                        wkt = wkp.tile([128, 256], BF16, tag="wk", name=f"wk{tb}_{kw}_{ct}")
                        nc.sync.dma_start(wkt[:], wkT[ct * 128 : (ct + 1) * 128, kw * 256 : (kw + 1) * 256])
                        for i in range(2):
                            nc.tensor.matmul(
                                kps[i][:],
                                wkt[:, i * 128 : (i + 1) * 128],
                                xb[ct][:],
                                start=(ct == 0),
                                stop=(ct == NCT - 1),
                            )
                    for i in range(2):
                        _rope(nc, tmpp, kT[kw * 2 + i][:, tsl], kps[i], cosb, nsinb)

                # V projection in [t, d] layout
                for vw in range(2):
                    vps = [projp.tile([128, NKV * HD], F32, tag="pp", name=f"vps{tb}_{vw}_{i}") for i in range(2)]
                    for ct in range(NCT):
                        wvt = wvp.tile([128, NKV * HD], BF16, tag="wv", name=f"wv{tb}_{vw}_{ct}")
                        nc.sync.dma_start(wvt[:], wvT[ct * 128 : (ct + 1) * 128, :])
                        for i in range(2):
                            nc.tensor.matmul(
                                vps[i][:],
                                xb[ct][:, (vw * 2 + i) * 128 : (vw * 2 + i + 1) * 128],
                                wvt[:],
                                start=(ct == 0),
                                stop=(ct == NCT - 1),
                            )
                    for i in range(2):
                        nc.vector.tensor_copy(vT[4 * tb + vw * 2 + i][:], vps[i][:])

                # Q projection (q^T layout) + RoPE, two waves of 4
                qts = []
                for wave in range(4):
                    qps = [projp.tile([128, TB], F32, tag="pp", name=f"qps{tb}_{wave}_{i}") for i in range(2)]
                    for ct in range(NCT):
                        wqt = wqp.tile([128, 256], BF16, tag="wq", name=f"wq{tb}_{wave}_{ct}")
                        nc.sync.dma_start(
                            wqt[:],
                            wqT[ct * 128 : (ct + 1) * 128, wave * 256 : (wave + 1) * 256],
                        )
                        for o in range(2):
                            nc.tensor.matmul(
                                qps[o][:],
                                wqt[:, o * 128 : (o + 1) * 128],
                                xb[ct][:],
                                start=(ct == 0),
                                stop=(ct == NCT - 1),
                            )
                    for o in range(2):
                        qt = qp.tile([HD, TB], BF16, tag="qt", name=f"qt{tb}_{wave}_{o}")
                        _rope(nc, tmpp, qt[:], qps[o], cosb, nsinb)
                        qts.append(qt)
                return qts

            def attention_block(tb, qts):
                ktmax = 4 * tb + 4
                outs = []
                tri = mask_t[0]  # [:, 0:128] is the lower-tri diagonal mask
                for h in range(NQH):
                    hv = h // 2
                    ops_ = opsum.tile([HD, TB], F32, tag="op", name=f"aop{tb}_{h}")
                    den = opsum.tile([1, TB], F32, tag="op", name=f"den{tb}_{h}")
                    for kt in range(ktmax):
                        sps = spsum.tile([128, TB], F32, tag="sp")
                        nc.tensor.matmul(
                            sps[:],
                            kT[hv][:, kt * 128 : (kt + 1) * 128],
                            qts[h][:],
                            start=True,
                            stop=True,
                        )
                        ex = expp.tile([128, TB], BF16, tag="exps")
                        nc.scalar.activation(ex[:], sps[:], EXP, scale=SCALE)
                        m = kt - 4 * tb
                        if m >= 0:
                            # causal: zero fully-masked q-subtiles (idle GpSimd)
                            # and apply the triangular mask on the diagonal one
                            if m > 0:
                                nc.gpsimd.memset(ex[:, 0 : 128 * m], 0.0)
                            nc.vector.tensor_mul(
                                ex[:, 128 * m : 128 * (m + 1)],
                                ex[:, 128 * m : 128 * (m + 1)],
                                tri[:, 0:128],
                            )
                        # denominator: accumulate ones.T @ ex on the PE in psum
                        nc.tensor.matmul(
                            den[:],
                            ones_c[:],
                            ex[:],
                            start=(kt == 0),
                            stop=(kt == ktmax - 1),
                        )
                        nc.tensor.matmul(
                            ops_[:],
                            vT[kt][:, hv * 128 : (hv + 1) * 128],
                            ex[:],
                            start=(kt == 0),
                            stop=(kt == ktmax - 1),
                        )
                    # single-op approx reciprocal (~18 bits, plenty), then
                    # partition-broadcast on the otherwise idle GpSimd engine
                    rec = smallp.tile([1, TB], F32, tag="rec")
                    nc.vector.reciprocal_approx_fast(rec[:], den[:])
                    bcs = smallp.tile([128, TB], F32, tag="bcs")
                    nc.gpsimd.partition_broadcast(bcs[:], rec[0:1, :])
                    ot = outp.tile([HD, TB], BF16, tag="ot")
                    nc.vector.tensor_mul(ot[:], ops_[:], bcs[:])
                    outs.append(ot)
                return outs

            def wo_block(tb, outs):
                tsl = slice(tb * TB, (tb + 1) * TB)
                for c2 in range(8):
                    yps = [projp.tile([128, TB], F32, tag="pp", name=f"yps{tb}_{c2}_{i}") for i in range(2)]
                    for jh in range(NQH):
                        wot = wop.tile([128, 256], BF16, tag="wo", name=f"wo{tb}_{c2}_{jh}")
                        nc.sync.dma_start(
                            wot[:],
                            woT[jh * 128 : (jh + 1) * 128, c2 * 256 : (c2 + 1) * 256],
                        )
                        for o in range(2):
                            nc.tensor.matmul(
                                yps[o][:],
                                wot[:, o * 128 : (o + 1) * 128],
                                outs[jh][:],
                                start=(jh == 0),
                                stop=(jh == NQH - 1),
                            )
                    for o in range(2):
                        ysb = yp.tile([128, TB], F32, tag="ysb")
                        nc.scalar.copy(ysb[:], yps[o][:])
                        og = c2 * 2 + o
                        nc.sync.dma_start(yT[og * 128 : (og + 1) * 128, tsl], ysb[:])

            # Software pipeline: attention/Wo of block tb-1 are emitted BEFORE
            # the projections of block tb, so the ACT-gated attention phase
            # always has dense projection matmuls to fill PE gaps (keeps the
            # HAM clock gate warm).
            prev_qts = None
            for tb in range(NTB):
                xb, cosb, nsinb = load_block(tb)
                if prev_qts is not None:
                    outs = attention_block(tb - 1, prev_qts)
                    wo_block(tb - 1, outs)
                prev_qts = proj_block(tb, xb, cosb, nsinb)
            outs = attention_block(NTB - 1, prev_qts)
            wo_block(NTB - 1, outs)

    nc.compile()
    return nc


def _host_consts():
    inv_freq = 1.0 / (10000.0 ** (np.arange(0, HD, 2, dtype=np.float32) / HD))
    t = np.arange(T, dtype=np.float32)
    freqs = np.outer(t, inv_freq)  # [T, HD/2]
    freqs = np.repeat(freqs, 2, axis=-1)  # [T, HD]
    cos = np.cos(freqs).astype(np.float32).T.copy()  # [HD, T]
    sin = np.sin(freqs).astype(np.float32).T.copy()
    # rotated-by-64 signed sin table: row d holds the multiplier that pairs
    # with x[(d+64)%128]; rows 64..127 carry -sin[0:64], rows 0..63 +sin[64:128]
    nsin = np.empty_like(sin)
    nsin[0:64, :] = sin[64:128, :]
    nsin[64:128, :] = -sin[0:64, :]

    masks = np.zeros((4, 128, TB), dtype=ml_dtypes.bfloat16)
    kp = np.arange(128)[:, None]
    qf = np.arange(TB)[None, :]
    for m in range(4):
        vis = (qf // 128 > m) | ((qf // 128 == m) & (kp <= qf % 128))
        masks[m] = vis.astype(ml_dtypes.bfloat16)

    return {
        "cosdt": np.ascontiguousarray(cos),
        "nsindt": np.ascontiguousarray(nsin),
        "masks": masks,
        "onescol": np.ones((128, 1), dtype=ml_dtypes.bfloat16),
        "onesrow": np.ones((1, 128), dtype=np.float32),
    }


_NC_CACHE = None


def _get_nc():
    global _NC_CACHE
    if _NC_CACHE is None:
        _NC_CACHE = build_nc()
    return _NC_CACHE


def kernel(x, Wq, Wk, Wv, Wo, _trace=False):
    x = np.asarray(x, dtype=np.float32)
    Wq = np.asarray(Wq, dtype=np.float32)
    Wk = np.asarray(Wk, dtype=np.float32)
    Wv = np.asarray(Wv, dtype=np.float32)
    Wo = np.asarray(Wo, dtype=np.float32)

    nc = _get_nc()
    consts = _host_consts()

    bf = ml_dtypes.bfloat16
    xTs = [np.ascontiguousarray(x[b].T.astype(bf)) for b in range(B)]
    wqTs = [np.ascontiguousarray(Wq[1024 * g : 1024 * (g + 1), :].T.astype(bf)) for g in range(2)]
    wkTs = [np.ascontiguousarray(Wk[512 * g : 512 * (g + 1), :].T.astype(bf)) for g in range(2)]
    wvTs = [np.ascontiguousarray(Wv[512 * g : 512 * (g + 1), :].T.astype(bf)) for g in range(2)]
    woTs = [np.ascontiguousarray(Wo[:, 1024 * g : 1024 * (g + 1)].T.astype(bf)) for g in range(2)]

    in_maps = []
    for c in range(8):
        b, g = c // 2, c % 2
        im = {
            "xT": xTs[b],
            "wqT": wqTs[g],
            "wkT": wkTs[g],
            "wvT": wvTs[g],
            "woT": woTs[g],
        }
        im.update(consts)
        in_maps.append(im)

    res = run_bass_kernel_spmd(nc, in_maps, core_ids=list(range(8)), trace=_trace)

    y = np.empty((B, T, C), dtype=np.float32)
    for b in range(B):
        y[b] = (res.results[2 * b]["yT"] + res.results[2 * b + 1]["yT"]).T
    if _trace:
        return y, res
    return y



# revision 4
# speedup vs baseline: 2.0670x; 2.0670x over previous
"""Causal GQA self-attention (B=4, T=2048, C=2048, 16 Q heads / 8 KV heads,
hd=128) as a Bass/Tile SPMD kernel on 8 Trainium2 NeuronCores.

Sharding: core c = (batch b = c//2, head-group g = c%2). Each core handles one
batch and 8 Q heads / 4 KV heads. Wq/Wk/Wv column-sharded on the head dim, Wo
row-sharded; the host sums the two partial Wo products per batch.

Layouts (all transposed [feature, token] so every contraction is on the
partition dim): qT/kT = [d, t], v = [t, d], scores as S^T = [k, q],
output y^T = [o, t]. Matmuls in bf16 with fp32 PSUM accumulation.

Schedule: one continuous PE instruction stream per token block. ACT-gated
attention steps (score matmul -> exp -> masked diag -> DVE denominator
accumulate -> out matmul, with a 2-step software-pipeline lookahead) are
interleaved one-for-one with dense projection/Wo matmul "fillers" so the PE
never waits on the ScalarE exp latency and the HAM clock gate stays at 8/8.
All weights are SBUF-persistent (loaded once). The softmax denominator is
accumulated on the DVE in fp16 (4x perf mode) and reduced+broadcast across
partitions on the otherwise-idle GpSimd, so the PE spends zero cycles on it.
The causal diagonal is trimmed: fully-masked 128-column subtiles are never
computed by the score/exp/out stages.
"""

import sys
from collections import deque

import ml_dtypes
import numpy as np

sys.path.insert(0, "/opt/trn_rl_repo")

import concourse.bass as bass  # noqa: E402
import concourse.mybir as mybir  # noqa: E402
import concourse.tile as tile  # noqa: E402
from concourse import bacc, bass_isa  # noqa: E402
from concourse.bass_utils import run_bass_kernel_spmd  # noqa: E402

# Problem shape (hardcoded per contest contract).
B = 4
T = 2048
C = 2048
HD = 128
N_HEAD = 16
N_KV_HEAD = 8
NQH = N_HEAD // 2  # q heads per core
NKV = N_KV_HEAD // 2  # kv heads per core
TB = 512  # token block
NTB = T // TB
NCT = C // 128  # contraction tiles for the projections
NOG = C // 128  # output row tiles for Wo
SCALE = 1.0 / float(np.sqrt(HD))
LOOKAHEAD = 2

F32 = mybir.dt.float32
F16 = mybir.dt.float16
BF16 = mybir.dt.bfloat16
MULT = mybir.AluOpType.mult
ADD = mybir.AluOpType.add
EXP = mybir.ActivationFunctionType.Exp


def build_nc():
    nc = bacc.Bacc("TRN2", target_bir_lowering=False, debug=False, num_devices=8)

    xT = nc.dram_tensor("xT", [C, T], BF16, kind="ExternalInput")
    wqT = nc.dram_tensor("wqT", [C, NQH * HD], BF16, kind="ExternalInput")
    wkT = nc.dram_tensor("wkT", [C, NKV * HD], BF16, kind="ExternalInput")
    wvT = nc.dram_tensor("wvT", [C, NKV * HD], BF16, kind="ExternalInput")
    woT = nc.dram_tensor("woT", [NQH * HD, C], BF16, kind="ExternalInput")
    cosdt = nc.dram_tensor("cosdt", [HD, T], BF16, kind="ExternalInput")
    nsindt = nc.dram_tensor("nsindt", [HD, T], BF16, kind="ExternalInput")
    tridt = nc.dram_tensor("tridt", [128, 128], BF16, kind="ExternalInput")
    yT = nc.dram_tensor("yT", [C, T], F32, kind="ExternalOutput")

    from contextlib import ExitStack

    with ExitStack() as es:
        tc = es.enter_context(tile.TileContext(nc))
        es.enter_context(nc.allow_low_precision("bf16 attention"))
        constp = es.enter_context(tc.tile_pool(name="const", bufs=1))
        perp = es.enter_context(tc.tile_pool(name="persist", bufs=1))
        strp = es.enter_context(tc.tile_pool(name="stream", bufs=2))
        xp = es.enter_context(tc.tile_pool(name="xp", bufs=32))
        qp = es.enter_context(tc.tile_pool(name="qt", bufs=8))
        expp = es.enter_context(tc.tile_pool(name="exs", bufs=4))
        exsump = es.enter_context(tc.tile_pool(name="exsum", bufs=2))
        sump = es.enter_context(tc.tile_pool(name="sums", bufs=1))
        outp = es.enter_context(tc.tile_pool(name="ot", bufs=16))
        tmpp = es.enter_context(tc.tile_pool(name="tmp", bufs=2))
        ysbp = es.enter_context(tc.tile_pool(name="ysb", bufs=2))
        projp = es.enter_context(tc.tile_pool(name="pp", bufs=3, space="PSUM"))
        spsum = es.enter_context(tc.tile_pool(name="sp", bufs=3, space="PSUM"))
        opsum = es.enter_context(tc.tile_pool(name="op", bufs=2, space="PSUM"))

        # ---- persistent tiles ----
        tri = constp.tile([128, 128], BF16, tag="tri", name="tri")
        wkt = [perp.tile([128, NKV * HD], BF16, tag=f"wk{ct}", name=f"wk{ct}") for ct in range(NCT)]
        wvt = [perp.tile([128, NKV * HD], BF16, tag=f"wv{ct}", name=f"wv{ct}") for ct in range(NCT)]
        wqt = [perp.tile([128, NQH * HD], BF16, tag=f"wq{ct}", name=f"wq{ct}") for ct in range(NCT)]
        wot = [perp.tile([128, C], BF16, tag=f"wo{jh}", name=f"wo{jh}") for jh in range(NQH)]
        kT = [perp.tile([HD, T], BF16, tag=f"kT{h}", name=f"kTt{h}") for h in range(NKV)]
        vT = [perp.tile([128, NKV * HD], BF16, tag=f"v{i}", name=f"vt{i}") for i in range(T // 128)]

        # ---- prologue DMAs (ordered so K-proj of block 0 can start ASAP) ----
        nc.sync.dma_start(tri[:], tridt[:])
        cos_t = {}
        nsin_t = {}
        cb = strp.tile([HD, TB], BF16, tag="cosb", name="cosb0")
        nc.sync.dma_start(cb[:], cosdt[:, 0:TB])
        cos_t[0] = cb
        nb = strp.tile([HD, TB], BF16, tag="nsinb", name="nsinb0")
        nc.sync.dma_start(nb[:], nsindt[:, 0:TB])
        nsin_t[0] = nb
        xb_t = {}  # (tb, ct) -> tile
        for ct in range(NCT):
            nc.sync.dma_start(wkt[ct][:], wkT[ct * 128 : (ct + 1) * 128, :])
            t_ = xp.tile([128, TB], BF16, tag="xb", name=f"xb0_{ct}")
            nc.sync.dma_start(t_[:], xT[ct * 128 : (ct + 1) * 128, 0:TB])
            xb_t[(0, ct)] = t_
        for ct in range(NCT):
            nc.sync.dma_start(wvt[ct][:], wvT[ct * 128 : (ct + 1) * 128, :])
        for ct in range(NCT):
            nc.sync.dma_start(wqt[ct][:], wqT[ct * 128 : (ct + 1) * 128, :])
        for jh in range(NQH):
            nc.sync.dma_start(wot[jh][:], woT[jh * 128 : (jh + 1) * 128, :])

        # ---- shared emission helpers ----
        qts_t = {}  # (tb, h) -> tile
        outs_t = {}  # (tb, h) -> tile
        emitted = set()

        def rope(dst, src_psum, tb):
            """dst = src*cos + rot_half(src)*sin, [d, t] layout; nsin table is
            pre-rotated by 64 partitions with sign folded in, so both halves
            are plain multiplies with aligned input base partitions."""
            cosb, nsinb = cos_t[tb], nsin_t[tb]
            t0 = tmpp.tile([HD, TB], BF16, tag="t0", name="ropet0")
            nc.scalar.copy(t0[:], src_psum[:])
            nc.vector.scalar_tensor_tensor(dst, t0[:], 1.0, cosb[:], op0=MULT, op1=MULT)
            t2 = tmpp.tile([HD, TB], BF16, tag="t2", name="ropet2")
            nc.vector.scalar_tensor_tensor(
                t2[0:64, :], t0[64:128, :], 1.0, nsinb[64:128, :], op0=MULT, op1=MULT
            )
            nc.vector.scalar_tensor_tensor(
                t2[64:128, :], t0[0:64, :], 1.0, nsinb[0:64, :], op0=MULT, op1=MULT
            )
            nc.vector.scalar_tensor_tensor(dst, t2[:], 1.0, dst, op0=MULT, op1=ADD)

        def build_fillers(tb):
            """Dense PE work for segment tb: K/V/Q projections of block tb,
            Wo of block tb-1, plus DMA prefetches for block tb+1. Each entry
            is (emit_fn, tag_or_None)."""
            fillers = []
            tsl = slice(tb * TB, (tb + 1) * TB)

            # K projection + RoPE -> kT[kv][:, tsl]
            for kv in range(NKV):
                kps = projp.tile([128, TB], F32, tag="pp", name=f"kps{tb}_{kv}")
                for ct in range(NCT):
                    def mk(kps=kps, kv=kv, ct=ct, tb=tb):
                        nc.tensor.matmul(
                            kps[:],
                            wkt[ct][:, kv * 128 : (kv + 1) * 128],
                            xb_t[(tb, ct)][:],
                            start=(ct == 0),
                            stop=(ct == NCT - 1),
                        )
                    fillers.append((mk, None))
                def mkr(kps=kps, kv=kv, tb=tb, tsl=tsl):
                    rope(kT[kv][:, tsl], kps, tb)
                fillers.append((mkr, ("k", tb, kv)))

            # V projection ([t, d] layout) -> vT[4*tb + i]
            for i in range(4):
                vps = projp.tile([128, NKV * HD], F32, tag="pp", name=f"vps{tb}_{i}")
                for ct in range(NCT):
                    def mv(vps=vps, i=i, ct=ct, tb=tb):
                        nc.tensor.matmul(
                            vps[:],
                            xb_t[(tb, ct)][:, i * 128 : (i + 1) * 128],
                            wvt[ct][:],
                            start=(ct == 0),
                            stop=(ct == NCT - 1),
                        )
                    fillers.append((mv, None))
                def mvc(vps=vps, i=i, tb=tb):
                    nc.vector.tensor_copy(vT[4 * tb + i][:], vps[:])
                fillers.append((mvc, ("v", tb, i)))

            # Q projection + RoPE -> qts, with x/cos prefetch DMAs sprinkled in
            for qh in range(NQH):
                qps = projp.tile([128, TB], F32, tag="pp", name=f"qps{tb}_{qh}")
                for ct in range(NCT):
                    def mq(qps=qps, qh=qh, ct=ct, tb=tb):
                        nc.tensor.matmul(
                            qps[:],
                            wqt[ct][:, qh * 128 : (qh + 1) * 128],
                            xb_t[(tb, ct)][:],
                            start=(ct == 0),
                            stop=(ct == NCT - 1),
                        )
                    fillers.append((mq, None))
                def mqr(qps=qps, qh=qh, tb=tb):
                    qt = qp.tile([HD, TB], BF16, tag="qt", name=f"qt{tb}_{qh}")
                    rope(qt[:], qps, tb)
                    qts_t[(tb, qh)] = qt
                fillers.append((mqr, ("q", tb, qh)))
                if tb + 1 < NTB:
                    ntsl = slice((tb + 1) * TB, (tb + 2) * TB)
                    for ct in (2 * qh, 2 * qh + 1):
                        def mdx(ct=ct, tb=tb, ntsl=ntsl):
                            t_ = xp.tile([128, TB], BF16, tag="xb", name=f"xb{tb+1}_{ct}")
                            nc.sync.dma_start(t_[:], xT[ct * 128 : (ct + 1) * 128, ntsl])
                            xb_t[(tb + 1, ct)] = t_
                        fillers.append((mdx, None))
                    if qh == 0:
                        def mdc(tb=tb, ntsl=ntsl):
                            cb = strp.tile([HD, TB], BF16, tag="cosb", name=f"cosb{tb+1}")
                            nc.sync.dma_start(cb[:], cosdt[:, ntsl])
                            cos_t[tb + 1] = cb
                            nb = strp.tile([HD, TB], BF16, tag="nsinb", name=f"nsinb{tb+1}")
                            nc.sync.dma_start(nb[:], nsindt[:, ntsl])
                            nsin_t[tb + 1] = nb
                        fillers.append((mdc, None))

            # Wo of block tb-1
            if tb > 0:
                fillers.extend(build_wo_fillers(tb - 1))
            return fillers

        def build_wo_fillers(wtb):
            fillers = []
            tsl = slice(wtb * TB, (wtb + 1) * TB)
            for og in range(NOG):
                yps = projp.tile([128, TB], F32, tag="pp", name=f"yps{wtb}_{og}")
                for jh in range(NQH):
                    def mw(yps=yps, og=og, jh=jh, wtb=wtb):
                        nc.tensor.matmul(
                            yps[:],
                            wot[jh][:, og * 128 : (og + 1) * 128],
                            outs_t[(wtb, jh)][:],
                            start=(jh == 0),
                            stop=(jh == NQH - 1),
                        )
                    fillers.append((mw, None))
                def mwc(yps=yps, og=og, tsl=tsl):
                    ysb = ysbp.tile([128, TB], F32, tag="ysb", name="ysb")
                    nc.vector.tensor_copy(ysb[:], yps[:])
                    nc.sync.dma_start(yT[og * 128 : (og + 1) * 128, tsl], ysb[:])
                fillers.append((mwc, None))
            return fillers

        def build_steps(tb):
            """Attention steps for block tb: S^T = k x q per (head, k-block),
            causally trimmed at 128-col granularity."""
            steps = []
            ktmax = 4 * tb + 4
            for h in range(NQH):
                for kt in range(ktmax):
                    m = kt - 4 * tb
                    lo = 128 * m if m > 0 else 0
                    needs = [("q", tb, h), ("k", kt // 4, h // 2), ("v", kt // 4, kt % 4)]
                    steps.append(
                        dict(
                            tb=tb, h=h, kt=kt, m=m, lo=lo,
                            first=(kt == 0), last=(kt == ktmax - 1),
                            needs=needs, ex=None,
                        )
                    )
            return steps

        head_state = {}  # h -> (ops_, exsum)

        def emit_score_phase(s):
            tb, h, kt, lo = s["tb"], s["h"], s["kt"], s["lo"]
            hv = h // 2
            if s["first"]:
                ops_ = opsum.tile([HD, TB], F32, tag="op", name=f"aop{tb}_{h}")
                exsum = exsump.tile([128, TB], F16, tag="exsum", name="exsum")
                head_state[(tb, h)] = (ops_, exsum)
            _, exsum = head_state[(tb, h)]
            sps = spsum.tile([128, TB], F32, tag="sp", name="sps")
            nc.tensor.matmul(
                sps[:, lo:TB],
                kT[hv][:, kt * 128 : (kt + 1) * 128],
                qts_t[(tb, h)][:, lo:TB],
                start=True,
                stop=True,
            )
            ex = expp.tile([128, TB], BF16, tag="ex", name="ex")
            nc.scalar.activation(ex[:, lo:TB], sps[:, lo:TB], EXP, scale=SCALE)
            if s["m"] >= 0:
                dcs = slice(128 * s["m"], 128 * (s["m"] + 1))
                nc.vector.scalar_tensor_tensor(
                    ex[:, dcs], ex[:, dcs], 1.0, tri[:], op0=MULT, op1=MULT
                )
            if s["first"]:
                nc.vector.tensor_copy(exsum[:], ex[:])
            else:
                nc.vector.scalar_tensor_tensor(
                    exsum[:, lo:TB], ex[:, lo:TB], 1.0, exsum[:, lo:TB],
                    op0=MULT, op1=ADD,
                )
            s["ex"] = ex

        def emit_out_phase(s):
            tb, h, kt, lo = s["tb"], s["h"], s["kt"], s["lo"]
            hv = h // 2
            ops_, exsum = head_state[(tb, h)]
            nc.tensor.matmul(
                ops_[:, lo:TB],
                vT[kt][:, hv * 128 : (hv + 1) * 128],
                s["ex"][:, lo:TB],
                start=s["first"],
                stop=s["last"],
            )
            if s["last"]:
                sums = sump.tile([128, TB], F32, tag="sums", name="sums")
                nc.gpsimd.partition_all_reduce(
                    sums[:], exsum[:], channels=128, reduce_op=bass_isa.ReduceOp.add
                )
                nc.vector.reciprocal_approx_fast(sums[:], sums[:])
                ot = outp.tile([HD, TB], BF16, tag="ot", name=f"ot{tb}_{h}")
                nc.vector.scalar_tensor_tensor(
                    ot[:], ops_[:], 1.0, sums[:], op0=MULT, op1=MULT
                )
                outs_t[(tb, h)] = ot

        def run_segment(steps, fillers):
            fc = 0

            def emit_filler():
                nonlocal fc
                fn, tag = fillers[fc]
                fn()
                if tag is not None:
                    emitted.add(tag)
                fc += 1

            pend = deque()
            for s in steps:
                while fc < len(fillers) and not all(t in emitted for t in s["needs"]):
                    emit_filler()
                emit_score_phase(s)
                pend.append(s)
                if len(pend) > LOOKAHEAD:
                    emit_out_phase(pend.popleft())
                if fc < len(fillers):
                    emit_filler()
            while pend:
                emit_out_phase(pend.popleft())
            while fc < len(fillers):
                emit_filler()

        for tb in range(NTB):
            run_segment(build_steps(tb), build_fillers(tb))
        run_segment([], build_wo_fillers(NTB - 1))

    nc.compile()
    return nc


def _host_consts():
    inv_freq = 1.0 / (10000.0 ** (np.arange(0, HD, 2, dtype=np.float32) / HD))
    t = np.arange(T, dtype=np.float32)
    freqs = np.outer(t, inv_freq)  # [T, HD/2]
    freqs = np.repeat(freqs, 2, axis=-1)  # [T, HD]
    cos = np.cos(freqs).astype(np.float32).T.copy()  # [HD, T]
    sin = np.sin(freqs).astype(np.float32).T.copy()
    # rotated-by-64 signed sin table: row d holds the multiplier that pairs
    # with x[(d+64)%128]; rows 64..127 carry -sin[0:64], rows 0..63 +sin[64:128]
    nsin = np.empty_like(sin)
    nsin[0:64, :] = sin[64:128, :]
    nsin[64:128, :] = -sin[0:64, :]

    bf = ml_dtypes.bfloat16
    kp = np.arange(128)[:, None]
    qf = np.arange(128)[None, :]
    tri = (kp <= qf).astype(bf)

    return {
        "cosdt": np.ascontiguousarray(cos.astype(bf)),
        "nsindt": np.ascontiguousarray(nsin.astype(bf)),
        "tridt": tri,
    }


_NC_CACHE = None


def _get_nc():
    global _NC_CACHE
    if _NC_CACHE is None:
        _NC_CACHE = build_nc()
    return _NC_CACHE


def kernel(x, Wq, Wk, Wv, Wo, _trace=False):
    x = np.asarray(x, dtype=np.float32)
    Wq = np.asarray(Wq, dtype=np.float32)
    Wk = np.asarray(Wk, dtype=np.float32)
    Wv = np.asarray(Wv, dtype=np.float32)
    Wo = np.asarray(Wo, dtype=np.float32)

    nc = _get_nc()
    consts = _host_consts()

    bf = ml_dtypes.bfloat16
    xTs = [np.ascontiguousarray(x[b].T.astype(bf)) for b in range(B)]
    wqTs = [np.ascontiguousarray(Wq[1024 * g : 1024 * (g + 1), :].T.astype(bf)) for g in range(2)]
    wkTs = [np.ascontiguousarray(Wk[512 * g : 512 * (g + 1), :].T.astype(bf)) for g in range(2)]
    wvTs = [np.ascontiguousarray(Wv[512 * g : 512 * (g + 1), :].T.astype(bf)) for g in range(2)]
    woTs = [np.ascontiguousarray(Wo[:, 1024 * g : 1024 * (g + 1)].T.astype(bf)) for g in range(2)]

    in_maps = []
    for c in range(8):
        b, g = c // 2, c % 2
        im = {
            "xT": xTs[b],
            "wqT": wqTs[g],
            "wkT": wkTs[g],
            "wvT": wvTs[g],
            "woT": woTs[g],
        }
        im.update(consts)
        in_maps.append(im)

    res = run_bass_kernel_spmd(nc, in_maps, core_ids=list(range(8)), trace=_trace)

    y = np.empty((B, T, C), dtype=np.float32)
    for b in range(B):
        y[b] = (res.results[2 * b]["yT"] + res.results[2 * b + 1]["yT"]).T
    if _trace:
        return y, res
    return y


# revision 11
# speedup vs baseline: 2.5982x; 1.2570x over previous
"""Causal GQA self-attention (B=4, T=2048, C=2048, 16 Q heads / 8 KV heads,
hd=128) as a Bass/Tile SPMD kernel on 8 Trainium2 NeuronCores.

Sharding: core c = (batch b = c//2, head-group g = c%2). Each core handles one
batch and 8 Q heads / 4 KV heads. Wq/Wk/Wv column-sharded on the head dim, Wo
row-sharded; the host sums the two partial Wo products per batch.

Layouts (all transposed [feature, token] so every contraction is on the
partition dim): qT/kT = [d, t], v = [t, d], scores as S^T = [k, q],
output y^T = [o, t]. Matmuls in bf16 with fp32 PSUM accumulation.

Schedule: one continuous PE instruction stream per token block. ACT-gated
attention steps (score matmul -> exp -> masked diag -> DVE denominator
accumulate -> out matmul, with a 2-step software-pipeline lookahead) are
interleaved one-for-one with dense projection/Wo matmul "fillers" so the PE
never waits on the ScalarE exp latency and the HAM clock gate stays at 8/8.
All weights are SBUF-persistent (loaded once). The softmax denominator is
accumulated on the DVE in fp16 (4x perf mode) and reduced+broadcast across
partitions on the otherwise-idle GpSimd, so the PE spends zero cycles on it.
The causal diagonal is trimmed: fully-masked 128-column subtiles are never
computed by the score/exp/out stages.
"""

import sys
from collections import deque

import ml_dtypes
import numpy as np

sys.path.insert(0, "/opt/trn_rl_repo")

import concourse.bass as bass  # noqa: E402
import concourse.mybir as mybir  # noqa: E402
import concourse.tile as tile  # noqa: E402
from concourse import bacc, bass_isa  # noqa: E402
from concourse.bass_utils import run_bass_kernel_spmd  # noqa: E402

# Problem shape (hardcoded per contest contract).
B = 4
T = 2048
C = 2048
HD = 128
N_HEAD = 16
N_KV_HEAD = 8
NQH = N_HEAD // 2  # q heads per core
NKV = N_KV_HEAD // 2  # kv heads per core
TB = 512  # token block
NTB = T // TB
NCT = C // 128  # contraction tiles for the projections
NOG = C // 128  # output row tiles for Wo
SCALE = 1.0 / float(np.sqrt(HD))
LOOKAHEAD = 2

F32 = mybir.dt.float32
F16 = mybir.dt.float16
BF16 = mybir.dt.bfloat16
MULT = mybir.AluOpType.mult
ADD = mybir.AluOpType.add
EXP = mybir.ActivationFunctionType.Exp


def build_nc():
    nc = bacc.Bacc("TRN2", target_bir_lowering=False, debug=False, num_devices=8)

    xT = nc.dram_tensor("xT", [C, T], BF16, kind="ExternalInput")
    wqT = nc.dram_tensor("wqT", [C, NQH * HD], BF16, kind="ExternalInput")
    wkT = nc.dram_tensor("wkT", [C, NKV * HD], BF16, kind="ExternalInput")
    wvT = nc.dram_tensor("wvT", [C, NKV * HD], BF16, kind="ExternalInput")
    woT = nc.dram_tensor("woT", [NQH * HD, C], BF16, kind="ExternalInput")
    cosdt = nc.dram_tensor("cosdt", [HD, T], BF16, kind="ExternalInput")
    nsindt = nc.dram_tensor("nsindt", [HD, T], BF16, kind="ExternalInput")
    tridt = nc.dram_tensor("tridt", [128, 128], BF16, kind="ExternalInput")
    onesdt = nc.dram_tensor("onesdt", [128, 128], F16, kind="ExternalInput")
    yT = nc.dram_tensor("yT", [C, T], F32, kind="ExternalOutput")

    from contextlib import ExitStack

    with ExitStack() as es:
        tc = es.enter_context(tile.TileContext(nc))
        es.enter_context(nc.allow_low_precision("bf16 attention"))
        constp = es.enter_context(tc.tile_pool(name="const", bufs=1))
        perp = es.enter_context(tc.tile_pool(name="persist", bufs=1))
        strp = es.enter_context(tc.tile_pool(name="stream", bufs=2))
        xp = es.enter_context(tc.tile_pool(name="xp", bufs=32))
        qp = es.enter_context(tc.tile_pool(name="qt", bufs=8))
        expp = es.enter_context(tc.tile_pool(name="exs", bufs=4))
        exsump = es.enter_context(tc.tile_pool(name="exsum", bufs=2))
        sump = es.enter_context(tc.tile_pool(name="sums", bufs=1))
        outp = es.enter_context(tc.tile_pool(name="ot", bufs=16))
        tmpp = es.enter_context(tc.tile_pool(name="tmp", bufs=2))
        ysbp = es.enter_context(tc.tile_pool(name="ysb", bufs=2))
        projp = es.enter_context(tc.tile_pool(name="pp", bufs=3, space="PSUM"))
        spsum = es.enter_context(tc.tile_pool(name="sp", bufs=3, space="PSUM"))
        opsum = es.enter_context(tc.tile_pool(name="op", bufs=2, space="PSUM"))

        # ---- persistent tiles ----
        tri = constp.tile([128, 128], BF16, tag="tri", name="tri")
        ones128 = constp.tile([128, 128], F16, tag="ones128", name="ones128")
        wkt = [perp.tile([128, NKV * HD], BF16, tag=f"wk{ct}", name=f"wk{ct}") for ct in range(NCT)]
        wvt = [perp.tile([128, NKV * HD], BF16, tag=f"wv{ct}", name=f"wv{ct}") for ct in range(NCT)]
        wqt = [perp.tile([128, NQH * HD], BF16, tag=f"wq{ct}", name=f"wq{ct}") for ct in range(NCT)]
        wot = [perp.tile([128, C], BF16, tag=f"wo{jh}", name=f"wo{jh}") for jh in range(NQH)]
        kT = [perp.tile([HD, T], BF16, tag=f"kT{h}", name=f"kTt{h}") for h in range(NKV)]
        vT = [perp.tile([128, NKV * HD], BF16, tag=f"v{i}", name=f"vt{i}") for i in range(T // 128)]

        # ---- prologue DMAs (ordered so K-proj of block 0 can start ASAP) ----
        nc.sync.dma_start(tri[:], tridt[:])
        nc.sync.dma_start(ones128[:], onesdt[:])
        cos_t = {}
        nsin_t = {}
        cb = strp.tile([HD, TB], BF16, tag="cosb", name="cosb0")
        nc.sync.dma_start(cb[:], cosdt[:, 0:TB])
        cos_t[0] = cb
        nb = strp.tile([HD, TB], BF16, tag="nsinb", name="nsinb0")
        nc.sync.dma_start(nb[:], nsindt[:, 0:TB])
        nsin_t[0] = nb
        xb_t = {}  # (tb, ct) -> tile
        for ct in range(NCT):
            nc.sync.dma_start(wkt[ct][:], wkT[ct * 128 : (ct + 1) * 128, :])
            t_ = xp.tile([128, TB], BF16, tag="xb", name=f"xb0_{ct}")
            nc.sync.dma_start(t_[:], xT[ct * 128 : (ct + 1) * 128, 0:TB])
            xb_t[(0, ct)] = t_
        for ct in range(NCT):
            nc.sync.dma_start(wvt[ct][:], wvT[ct * 128 : (ct + 1) * 128, :])
        for ct in range(NCT):
            nc.sync.dma_start(wqt[ct][:], wqT[ct * 128 : (ct + 1) * 128, :])
        for jh in range(NQH):
            nc.sync.dma_start(wot[jh][:], woT[jh * 128 : (jh + 1) * 128, :])

        # ---- shared emission helpers ----
        qts_t = {}  # (tb, h) -> tile
        outs_t = {}  # (tb, h) -> tile
        emitted = set()

        def rope(dst, src_psum, tb):
            """dst = src*cos + rot_half(src)*sin, [d, t] layout; nsin table is
            pre-rotated by 64 partitions with sign folded in, so both halves
            are plain multiplies with aligned input base partitions."""
            cosb, nsinb = cos_t[tb], nsin_t[tb]
            t0 = tmpp.tile([HD, TB], BF16, tag="t0", name="ropet0")
            nc.scalar.copy(t0[:], src_psum[:])
            nc.vector.tensor_mul(dst, t0[:], cosb[:])
            t2 = tmpp.tile([HD, TB], BF16, tag="t2", name="ropet2")
            nc.vector.tensor_mul(t2[0:64, :], t0[64:128, :], nsinb[64:128, :])
            nc.vector.tensor_mul(t2[64:128, :], t0[0:64, :], nsinb[0:64, :])
            nc.vector.tensor_add(dst, dst, t2[:])

        def build_fillers(tb):
            """Dense PE work for segment tb: K/V/Q projections of block tb,
            Wo of block tb-1, plus DMA prefetches for block tb+1. Each entry
            is (emit_fn, tag_or_None)."""
            fillers = []
            tsl = slice(tb * TB, (tb + 1) * TB)

            # K projection + RoPE -> kT[kv][:, tsl]
            for kv in range(NKV):
                kps = projp.tile([128, TB], F32, tag="pp", name=f"kps{tb}_{kv}")
                for ct in range(NCT):
                    def mk(kps=kps, kv=kv, ct=ct, tb=tb):
                        nc.tensor.matmul(
                            kps[:],
                            wkt[ct][:, kv * 128 : (kv + 1) * 128],
                            xb_t[(tb, ct)][:],
                            start=(ct == 0),
                            stop=(ct == NCT - 1),
                        )
                    fillers.append((mk, None))
                def mkr(kps=kps, kv=kv, tb=tb, tsl=tsl):
                    rope(kT[kv][:, tsl], kps, tb)
                fillers.append((mkr, ("k", tb, kv)))

            # V projection ([t, d] layout) -> vT[4*tb + i]
            for i in range(4):
                vps = projp.tile([128, NKV * HD], F32, tag="pp", name=f"vps{tb}_{i}")
                for ct in range(NCT):
                    def mv(vps=vps, i=i, ct=ct, tb=tb):
                        nc.tensor.matmul(
                            vps[:],
                            xb_t[(tb, ct)][:, i * 128 : (i + 1) * 128],
                            wvt[ct][:],
                            start=(ct == 0),
                            stop=(ct == NCT - 1),
                        )
                    fillers.append((mv, None))
                def mvc(vps=vps, i=i, tb=tb):
                    nc.vector.tensor_copy(vT[4 * tb + i][:], vps[:])
                fillers.append((mvc, ("v", tb, i)))

            # Q projection + RoPE -> qts, with x/cos prefetch DMAs sprinkled in
            for qh in range(NQH):
                qps = projp.tile([128, TB], F32, tag="pp", name=f"qps{tb}_{qh}")
                for ct in range(NCT):
                    def mq(qps=qps, qh=qh, ct=ct, tb=tb):
                        nc.tensor.matmul(
                            qps[:],
                            wqt[ct][:, qh * 128 : (qh + 1) * 128],
                            xb_t[(tb, ct)][:],
                            start=(ct == 0),
                            stop=(ct == NCT - 1),
                        )
                    fillers.append((mq, None))
                def mqr(qps=qps, qh=qh, tb=tb):
                    qt = qp.tile([HD, TB], BF16, tag="qt", name=f"qt{tb}_{qh}")
                    rope(qt[:], qps, tb)
                    qts_t[(tb, qh)] = qt
                fillers.append((mqr, ("q", tb, qh)))
                if tb + 1 < NTB:
                    ntsl = slice((tb + 1) * TB, (tb + 2) * TB)
                    for ct in (2 * qh, 2 * qh + 1):
                        def mdx(ct=ct, tb=tb, ntsl=ntsl):
                            t_ = xp.tile([128, TB], BF16, tag="xb", name=f"xb{tb+1}_{ct}")
                            nc.sync.dma_start(t_[:], xT[ct * 128 : (ct + 1) * 128, ntsl])
                            xb_t[(tb + 1, ct)] = t_
                        fillers.append((mdx, None))
                    if qh == 0:
                        def mdc(tb=tb, ntsl=ntsl):
                            cb = strp.tile([HD, TB], BF16, tag="cosb", name=f"cosb{tb+1}")
                            nc.sync.dma_start(cb[:], cosdt[:, ntsl])
                            cos_t[tb + 1] = cb
                            nb = strp.tile([HD, TB], BF16, tag="nsinb", name=f"nsinb{tb+1}")
                            nc.sync.dma_start(nb[:], nsindt[:, ntsl])
                            nsin_t[tb + 1] = nb
                        fillers.append((mdc, None))

            # Wo of block tb-1
            if tb > 0:
                fillers.extend(build_wo_fillers(tb - 1))
            return fillers

        def build_wo_fillers(wtb):
            fillers = []
            tsl = slice(wtb * TB, (wtb + 1) * TB)
            for og in range(NOG):
                yps = projp.tile([128, TB], F32, tag="pp", name=f"yps{wtb}_{og}")
                for jh in range(NQH):
                    def mw(yps=yps, og=og, jh=jh, wtb=wtb):
                        nc.tensor.matmul(
                            yps[:],
                            wot[jh][:, og * 128 : (og + 1) * 128],
                            outs_t[(wtb, jh)][:],
                            start=(jh == 0),
                            stop=(jh == NQH - 1),
                        )
                    fillers.append((mw, None))
                def mwc(yps=yps, og=og, tsl=tsl):
                    ysb = ysbp.tile([128, TB], F32, tag="ysb", name="ysb")
                    nc.vector.tensor_copy(ysb[:], yps[:])
                    nc.sync.dma_start(yT[og * 128 : (og + 1) * 128, tsl], ysb[:])
                fillers.append((mwc, None))
            return fillers

        def build_steps(tb):
            """Attention steps for block tb: S^T = k x q per (head, k-block),
            causally trimmed at 128-col granularity."""
            steps = []
            ktmax = 4 * tb + 4
            for h in range(NQH):
                for kt in range(ktmax):
                    m = kt - 4 * tb
                    lo = 128 * m if m > 0 else 0
                    needs = [("q", tb, h), ("k", kt // 4, h // 2), ("v", kt // 4, kt % 4)]
                    steps.append(
                        dict(
                            tb=tb, h=h, kt=kt, m=m, lo=lo,
                            first=(kt == 0), last=(kt == ktmax - 1),
                            needs=needs, ex=None,
                        )
                    )
            return steps

        head_state = {}  # h -> (ops_, exsum)

        def emit_score_phase(s):
            tb, h, kt, lo = s["tb"], s["h"], s["kt"], s["lo"]
            hv = h // 2
            if s["first"]:
                ops_ = opsum.tile([HD, TB], F32, tag="op", name=f"aop{tb}_{h}")
                exsum = exsump.tile([128, TB], F16, tag="exsum", name="exsum")
                head_state[(tb, h)] = (ops_, exsum)
            _, exsum = head_state[(tb, h)]
            sps = spsum.tile([128, TB], F32, tag="sp", name="sps")
            nc.tensor.matmul(
                sps[:, lo:TB],
                kT[hv][:, kt * 128 : (kt + 1) * 128],
                qts_t[(tb, h)][:, lo:TB],
                start=True,
                stop=True,
            )
            ex = expp.tile([128, TB], BF16, tag="ex", name="ex")
            nc.scalar.activation(ex[:, lo:TB], sps[:, lo:TB], EXP, scale=SCALE)
            if s["m"] >= 0:
                dcs = slice(128 * s["m"], 128 * (s["m"] + 1))
                nc.vector.tensor_mul(ex[:, dcs], ex[:, dcs], tri[:])
            if s["first"]:
                nc.vector.tensor_copy(exsum[:], ex[:])
            else:
                nc.vector.tensor_add(exsum[:, lo:TB], ex[:, lo:TB], exsum[:, lo:TB])
            s["ex"] = ex

        def emit_out_phase(s):
            tb, h, kt, lo = s["tb"], s["h"], s["kt"], s["lo"]
            hv = h // 2
            ops_, exsum = head_state[(tb, h)]
            nc.tensor.matmul(
                ops_[:, lo:TB],
                vT[kt][:, hv * 128 : (hv + 1) * 128],
                s["ex"][:, lo:TB],
                start=s["first"],
                stop=s["last"],
            )
            if s["last"]:
                # softmax denominator: reduce over k partitions AND broadcast
                # to all 128 partitions in one PE matmul against a ones matrix
                den = spsum.tile([128, TB], F32, tag="sp", name="den")
                nc.tensor.matmul(den[:], ones128[:], exsum[:], start=True, stop=True)
                sums = sump.tile([128, TB], F32, tag="sums", name="sums")
                nc.vector.reciprocal_approx_fast(sums[:], den[:])
                ot = outp.tile([HD, TB], BF16, tag="ot", name=f"ot{tb}_{h}")
                nc.vector.tensor_mul(ot[:], ops_[:], sums[:])
                outs_t[(tb, h)] = ot

        def run_segment(steps, fillers):
            fc = 0

            def emit_filler():
                nonlocal fc
                fn, tag = fillers[fc]
                fn()
                if tag is not None:
                    emitted.add(tag)
                fc += 1

            pend = deque()
            for s in steps:
                while fc < len(fillers) and not all(t in emitted for t in s["needs"]):
                    emit_filler()
                emit_score_phase(s)
                pend.append(s)
                if len(pend) > LOOKAHEAD:
                    emit_out_phase(pend.popleft())
                if fc < len(fillers):
                    emit_filler()
            while pend:
                emit_out_phase(pend.popleft())
            while fc < len(fillers):
                emit_filler()

        for tb in range(NTB):
            run_segment(build_steps(tb), build_fillers(tb))
        run_segment([], build_wo_fillers(NTB - 1))

    nc.compile()
    return nc


def _host_consts():
    inv_freq = 1.0 / (10000.0 ** (np.arange(0, HD, 2, dtype=np.float32) / HD))
    t = np.arange(T, dtype=np.float32)
    freqs = np.outer(t, inv_freq)  # [T, HD/2]
    freqs = np.repeat(freqs, 2, axis=-1)  # [T, HD]
    cos = np.cos(freqs).astype(np.float32).T.copy()  # [HD, T]
    sin = np.sin(freqs).astype(np.float32).T.copy()
    # rotated-by-64 signed sin table: row d holds the multiplier that pairs
    # with x[(d+64)%128]; rows 64..127 carry -sin[0:64], rows 0..63 +sin[64:128]
    nsin = np.empty_like(sin)
    nsin[0:64, :] = sin[64:128, :]
    nsin[64:128, :] = -sin[0:64, :]

    bf = ml_dtypes.bfloat16
    kp = np.arange(128)[:, None]
    qf = np.arange(128)[None, :]
    tri = (kp <= qf).astype(bf)

    return {
        "cosdt": np.ascontiguousarray(cos.astype(bf)),
        "nsindt": np.ascontiguousarray(nsin.astype(bf)),
        "tridt": tri,
        "onesdt": np.ones((128, 128), dtype=np.float16),
    }


_NC_CACHE = None


def _get_nc():
    global _NC_CACHE
    if _NC_CACHE is None:
        _NC_CACHE = build_nc()
    return _NC_CACHE


def kernel(x, Wq, Wk, Wv, Wo, _trace=False):
    x = np.asarray(x, dtype=np.float32)
    Wq = np.asarray(Wq, dtype=np.float32)
    Wk = np.asarray(Wk, dtype=np.float32)
    Wv = np.asarray(Wv, dtype=np.float32)
    Wo = np.asarray(Wo, dtype=np.float32)

    nc = _get_nc()
    consts = _host_consts()

    bf = ml_dtypes.bfloat16
    xTs = [np.ascontiguousarray(x[b].T.astype(bf)) for b in range(B)]
    wqTs = [np.ascontiguousarray(Wq[1024 * g : 1024 * (g + 1), :].T.astype(bf)) for g in range(2)]
    wkTs = [np.ascontiguousarray(Wk[512 * g : 512 * (g + 1), :].T.astype(bf)) for g in range(2)]
    wvTs = [np.ascontiguousarray(Wv[512 * g : 512 * (g + 1), :].T.astype(bf)) for g in range(2)]
    woTs = [np.ascontiguousarray(Wo[:, 1024 * g : 1024 * (g + 1)].T.astype(bf)) for g in range(2)]

    in_maps = []
    for c in range(8):
        b, g = c // 2, c % 2
        im = {
            "xT": xTs[b],
            "wqT": wqTs[g],
            "wkT": wkTs[g],
            "wvT": wvTs[g],
            "woT": woTs[g],
        }
        im.update(consts)
        in_maps.append(im)

    res = run_bass_kernel_spmd(nc, in_maps, core_ids=list(range(8)), trace=_trace)

    y = np.empty((B, T, C), dtype=np.float32)
    for b in range(B):
        y[b] = (res.results[2 * b]["yT"] + res.results[2 * b + 1]["yT"]).T
    if _trace:
        return y, res
    return y


# revision 15
# speedup vs baseline: 2.7042x; 1.0408x over previous
"""Causal GQA self-attention (B=4, T=2048, C=2048, 16 Q heads / 8 KV heads,
hd=128) as a Bass/Tile SPMD kernel on 8 Trainium2 NeuronCores.

Sharding: core c = (batch b = c//2, head-group g = c%2). Each core handles one
batch and 8 Q heads / 4 KV heads. Wq/Wk/Wv column-sharded on the head dim, Wo
row-sharded; the host sums the two partial Wo products per batch.

Layouts (all transposed [feature, token] so every contraction is on the
partition dim): qT/kT = [d, t], v = [t, d], scores as S^T = [k, q],
output y^T = [o, t]. Matmuls in bf16 with fp32 PSUM accumulation.

Schedule: one continuous PE instruction stream per token block. ACT-gated
attention steps (score matmul -> exp -> masked diag -> DVE denominator
accumulate -> out matmul, with a 2-step software-pipeline lookahead) are
interleaved one-for-one with dense projection/Wo matmul "fillers" so the PE
never waits on the ScalarE exp latency and the HAM clock gate stays at 8/8.
All weights are SBUF-persistent (loaded once). The softmax denominator is
accumulated on the DVE in fp16 (4x perf mode) and reduced+broadcast across
partitions on the otherwise-idle GpSimd, so the PE spends zero cycles on it.
The causal diagonal is trimmed: fully-masked 128-column subtiles are never
computed by the score/exp/out stages.
"""

import sys
from collections import deque

import ml_dtypes
import numpy as np

sys.path.insert(0, "/opt/trn_rl_repo")

import concourse.bass as bass  # noqa: E402
import concourse.mybir as mybir  # noqa: E402
import concourse.tile as tile  # noqa: E402
from concourse import bacc, bass_isa  # noqa: E402
from concourse.bass_utils import run_bass_kernel_spmd  # noqa: E402

# Problem shape (hardcoded per contest contract).
B = 4
T = 2048
C = 2048
HD = 128
N_HEAD = 16
N_KV_HEAD = 8
NQH = N_HEAD // 2  # q heads per core
NKV = N_KV_HEAD // 2  # kv heads per core
TB = 512  # token block
NTB = T // TB
NCT = C // 128  # contraction tiles for the projections
NOG = C // 128  # output row tiles for Wo
SCALE = 1.0 / float(np.sqrt(HD))
LOOKAHEAD = 2

F32 = mybir.dt.float32
F16 = mybir.dt.float16
BF16 = mybir.dt.bfloat16
MULT = mybir.AluOpType.mult
ADD = mybir.AluOpType.add
EXP = mybir.ActivationFunctionType.Exp


def build_nc():
    nc = bacc.Bacc("TRN2", target_bir_lowering=False, debug=False, num_devices=8)

    # packed partition-major layouts (host pre-transposed for wide DMA runs):
    # xb[p, (tb*NCT+ct)*TB + t'] = x[b].T[ct*128+p, tb*TB+t']
    xb_d = nc.dram_tensor("xb_d", [128, NTB * NCT * TB], BF16, kind="ExternalInput")
    # wq[p, qh*(NCT*128) + ct*128 + j] = Wq.T[ct*128+p, qh*128+j]  (head-major)
    wq_d = nc.dram_tensor("wq_d", [128, NQH * NCT * 128], BF16, kind="ExternalInput")
    # wk/wv[p, ct*512 + j] = W.T[ct*128+p, j]  (ct-major)
    wk_d = nc.dram_tensor("wk_d", [128, NCT * NKV * HD], BF16, kind="ExternalInput")
    wv_d = nc.dram_tensor("wv_d", [128, NCT * NKV * HD], BF16, kind="ExternalInput")
    # wo[p, jh*C + j] = Wo_part.T[jh*128+p, j]  (contraction-tile-major)
    wo_d = nc.dram_tensor("wo_d", [128, NQH * C], BF16, kind="ExternalInput")
    cosdt = nc.dram_tensor("cosdt", [HD, T], BF16, kind="ExternalInput")
    nsindt = nc.dram_tensor("nsindt", [HD, T], BF16, kind="ExternalInput")
    tridt = nc.dram_tensor("tridt", [128, 128], BF16, kind="ExternalInput")
    onesdt = nc.dram_tensor("onesdt", [128, 128], F16, kind="ExternalInput")
    yT = nc.dram_tensor("yT", [C, T], BF16, kind="ExternalOutput")

    from contextlib import ExitStack

    with ExitStack() as es:
        tc = es.enter_context(tile.TileContext(nc))
        es.enter_context(nc.allow_low_precision("bf16 attention"))
        constp = es.enter_context(tc.tile_pool(name="const", bufs=1))
        perp = es.enter_context(tc.tile_pool(name="persist", bufs=1))
        strp = es.enter_context(tc.tile_pool(name="stream", bufs=2))
        xp = es.enter_context(tc.tile_pool(name="xp", bufs=2))
        qp = es.enter_context(tc.tile_pool(name="qt", bufs=8))
        expp = es.enter_context(tc.tile_pool(name="exs", bufs=4))
        exsump = es.enter_context(tc.tile_pool(name="exsum", bufs=2))
        sump = es.enter_context(tc.tile_pool(name="sums", bufs=1))
        outp = es.enter_context(tc.tile_pool(name="ot", bufs=16))
        tmpp = es.enter_context(tc.tile_pool(name="tmp", bufs=2))
        ysbp = es.enter_context(tc.tile_pool(name="ysb", bufs=2))
        projp = es.enter_context(tc.tile_pool(name="pp", bufs=3, space="PSUM"))
        spsum = es.enter_context(tc.tile_pool(name="sp", bufs=3, space="PSUM"))
        opsum = es.enter_context(tc.tile_pool(name="op", bufs=2, space="PSUM"))

        # ---- persistent tiles ----
        tri = constp.tile([128, 128], BF16, tag="tri", name="tri")
        ones128 = constp.tile([128, 128], F16, tag="ones128", name="ones128")
        wk_all = perp.tile([128, NCT * NKV * HD], BF16, tag="wk", name="wk_all")
        wv_all = perp.tile([128, NCT * NKV * HD], BF16, tag="wv", name="wv_all")
        wq_all = perp.tile([128, NQH * NCT * 128], BF16, tag="wq", name="wq_all")
        wo_all = perp.tile([128, NQH * C], BF16, tag="wo", name="wo_all")
        dummy = constp.tile([128, 128], BF16, tag="dummy", name="dummy")
        kT = [perp.tile([HD, T], BF16, tag=f"kT{h}", name=f"kTt{h}") for h in range(NKV)]
        vT = [perp.tile([128, NKV * HD], BF16, tag=f"v{i}", name=f"vt{i}") for i in range(T // 128)]

        # ---- PE warm-up: dummy matmuls during the DMA prologue keep the
        # HAM activity window busy so real matmuls start at the 2.4 GHz
        # clock instead of the cold 1.2 GHz default.
        nc.gpsimd.memset(dummy[:], 0.0)
        wps = projp.tile([128, 128], F32, tag="pp", name="warmps")
        for _ in range(112):
            nc.tensor.matmul(wps[:], dummy[:], dummy[:], start=True, stop=True)

        # ---- prologue DMAs (ordered so K-proj of block 0 can start ASAP) ----
        nc.sync.dma_start(tri[:], tridt[:])
        nc.sync.dma_start(ones128[:], onesdt[:])
        cos_t = {}
        nsin_t = {}
        cb = strp.tile([HD, TB], BF16, tag="cosb", name="cosb0")
        nc.sync.dma_start(cb[:], cosdt[:, 0:TB])
        cos_t[0] = cb
        nb = strp.tile([HD, TB], BF16, tag="nsinb", name="nsinb0")
        nc.sync.dma_start(nb[:], nsindt[:, 0:TB])
        nsin_t[0] = nb
        xb_set = {}  # tb -> big tile [128, NCT*TB]
        xb_set[0] = xp.tile([128, NCT * TB], BF16, tag="xb", name="xb0")
        WKC = NCT * NKV * HD // 4
        XC = NCT * TB // 4
        for c4 in range(4):  # interleave wk / x(0) chunks
            nc.sync.dma_start(
                wk_all[:, c4 * WKC : (c4 + 1) * WKC], wk_d[:, c4 * WKC : (c4 + 1) * WKC]
            )
            nc.sync.dma_start(
                xb_set[0][:, c4 * XC : (c4 + 1) * XC], xb_d[:, c4 * XC : (c4 + 1) * XC]
            )
        for c4 in range(4):
            nc.sync.dma_start(
                wv_all[:, c4 * WKC : (c4 + 1) * WKC], wv_d[:, c4 * WKC : (c4 + 1) * WKC]
            )
        WQH = NCT * 128
        for qh in range(NQH):  # head-major so Q-proj streams incrementally
            nc.sync.dma_start(
                wq_all[:, qh * WQH : (qh + 1) * WQH], wq_d[:, qh * WQH : (qh + 1) * WQH]
            )
        for c4 in range(4):
            WOC = NQH * C // 4
            nc.sync.dma_start(
                wo_all[:, c4 * WOC : (c4 + 1) * WOC], wo_d[:, c4 * WOC : (c4 + 1) * WOC]
            )

        # ---- shared emission helpers ----
        qts_t = {}  # (tb, h) -> tile
        outs_t = {}  # (tb, h) -> tile
        emitted = set()

        def rope(dst, src_psum, tb):
            """dst = src*cos + rot_half(src)*sin, [d, t] layout; nsin table is
            pre-rotated by 64 partitions with sign folded in, so both halves
            are plain multiplies with aligned input base partitions."""
            cosb, nsinb = cos_t[tb], nsin_t[tb]
            t0 = tmpp.tile([HD, TB], BF16, tag="t0", name="ropet0")
            nc.scalar.copy(t0[:], src_psum[:])
            nc.vector.tensor_mul(dst, t0[:], cosb[:])
            t2 = tmpp.tile([HD, TB], BF16, tag="t2", name="ropet2")
            nc.vector.tensor_mul(t2[0:64, :], t0[64:128, :], nsinb[64:128, :])
            nc.vector.tensor_mul(t2[64:128, :], t0[0:64, :], nsinb[0:64, :])
            nc.vector.tensor_add(dst, dst, t2[:])

        def build_fillers(tb):
            """Dense PE work for segment tb: K/V/Q projections of block tb,
            Wo of block tb-1, plus DMA prefetches for block tb+1. Each entry
            is (emit_fn, tag_or_None)."""
            fillers = []
            tsl = slice(tb * TB, (tb + 1) * TB)

            # K projection + RoPE -> kT[kv][:, tsl]
            xb = xb_set[tb]
            for kv in range(NKV):
                kps = projp.tile([128, TB], F32, tag="pp", name=f"kps{tb}_{kv}")
                for ct in range(NCT):
                    def mk(kps=kps, kv=kv, ct=ct, xb=xb):
                        nc.tensor.matmul(
                            kps[:],
                            wk_all[:, ct * 512 + kv * 128 : ct * 512 + (kv + 1) * 128],
                            xb[:, ct * TB : (ct + 1) * TB],
                            start=(ct == 0),
                            stop=(ct == NCT - 1),
                        )
                    fillers.append((mk, None))
                def mkr(kps=kps, kv=kv, tb=tb, tsl=tsl):
                    rope(kT[kv][:, tsl], kps, tb)
                fillers.append((mkr, ("k", tb, kv)))

            # V projection ([t, d] layout) -> vT[4*tb + i]
            for i in range(4):
                vps = projp.tile([128, NKV * HD], F32, tag="pp", name=f"vps{tb}_{i}")
                for ct in range(NCT):
                    def mv(vps=vps, i=i, ct=ct, xb=xb):
                        nc.tensor.matmul(
                            vps[:],
                            xb[:, ct * TB + i * 128 : ct * TB + (i + 1) * 128],
                            wv_all[:, ct * 512 : (ct + 1) * 512],
                            start=(ct == 0),
                            stop=(ct == NCT - 1),
                        )
                    fillers.append((mv, None))
                def mvc(vps=vps, i=i, tb=tb):
                    nc.vector.tensor_copy(vT[4 * tb + i][:], vps[:])
                fillers.append((mvc, ("v", tb, i)))

            # Q projection + RoPE -> qts, with x/cos prefetch DMAs sprinkled in
            for qh in range(NQH):
                qps = projp.tile([128, TB], F32, tag="pp", name=f"qps{tb}_{qh}")
                for ct in range(NCT):
                    def mq(qps=qps, qh=qh, ct=ct, xb=xb):
                        nc.tensor.matmul(
                            qps[:],
                            wq_all[:, qh * 2048 + ct * 128 : qh * 2048 + (ct + 1) * 128],
                            xb[:, ct * TB : (ct + 1) * TB],
                            start=(ct == 0),
                            stop=(ct == NCT - 1),
                        )
                    fillers.append((mq, None))
                def mqr(qps=qps, qh=qh, tb=tb):
                    qt = qp.tile([HD, TB], BF16, tag="qt", name=f"qt{tb}_{qh}")
                    rope(qt[:], qps, tb)
                    qts_t[(tb, qh)] = qt
                fillers.append((mqr, ("q", tb, qh)))
                if tb + 1 < NTB and qh < 4:
                    def mdx(c4=qh, tb=tb):
                        if c4 == 0:
                            xb_set[tb + 1] = xp.tile(
                                [128, NCT * TB], BF16, tag="xb", name=f"xb{tb+1}"
                            )
                        XSEG = NCT * TB
                        XC = XSEG // 4
                        nc.sync.dma_start(
                            xb_set[tb + 1][:, c4 * XC : (c4 + 1) * XC],
                            xb_d[:, (tb + 1) * XSEG + c4 * XC : (tb + 1) * XSEG + (c4 + 1) * XC],
                        )
                    fillers.append((mdx, None))
                if tb + 1 < NTB and qh == 4:
                    ntsl = slice((tb + 1) * TB, (tb + 2) * TB)
                    def mdc(tb=tb, ntsl=ntsl):
                        cb = strp.tile([HD, TB], BF16, tag="cosb", name=f"cosb{tb+1}")
                        nc.sync.dma_start(cb[:], cosdt[:, ntsl])
                        cos_t[tb + 1] = cb
                        nb = strp.tile([HD, TB], BF16, tag="nsinb", name=f"nsinb{tb+1}")
                        nc.sync.dma_start(nb[:], nsindt[:, ntsl])
                        nsin_t[tb + 1] = nb
                    fillers.append((mdc, None))

            # Wo of block tb-1
            if tb > 0:
                fillers.extend(build_wo_fillers(tb - 1))
            return fillers

        def build_wo_fillers(wtb):
            fillers = []
            tsl = slice(wtb * TB, (wtb + 1) * TB)
            for og in range(NOG):
                yps = projp.tile([128, TB], F32, tag="pp", name=f"yps{wtb}_{og}")
                for jh in range(NQH):
                    def mw(yps=yps, og=og, jh=jh, wtb=wtb):
                        nc.tensor.matmul(
                            yps[:],
                            wo_all[:, jh * C + og * 128 : jh * C + (og + 1) * 128],
                            outs_t[(wtb, jh)][:],
                            start=(jh == 0),
                            stop=(jh == NQH - 1),
                        )
                    fillers.append((mw, None))
                def mwc(yps=yps, og=og, tsl=tsl):
                    ysb = ysbp.tile([128, TB], BF16, tag="ysb", name="ysb")
                    nc.vector.tensor_copy(ysb[:], yps[:])
                    nc.sync.dma_start(yT[og * 128 : (og + 1) * 128, tsl], ysb[:])
                fillers.append((mwc, None))
            return fillers

        def build_steps(tb):
            """Attention steps for block tb: S^T = k x q per (head, k-block),
            causally trimmed at 128-col granularity."""
            steps = []
            ktmax = 4 * tb + 4
            for h in range(NQH):
                for kt in range(ktmax):
                    m = kt - 4 * tb
                    lo = 128 * m if m > 0 else 0
                    needs = [("q", tb, h), ("k", kt // 4, h // 2), ("v", kt // 4, kt % 4)]
                    steps.append(
                        dict(
                            tb=tb, h=h, kt=kt, m=m, lo=lo,
                            first=(kt == 0), last=(kt == ktmax - 1),
                            needs=needs, ex=None,
                        )
                    )
            return steps

        head_state = {}  # h -> (ops_, exsum)

        def emit_score_phase(s):
            tb, h, kt, lo = s["tb"], s["h"], s["kt"], s["lo"]
            hv = h // 2
            if s["first"]:
                ops_ = opsum.tile([HD, TB], F32, tag="op", name=f"aop{tb}_{h}")
                exsum = exsump.tile([128, TB], F16, tag="exsum", name="exsum")
                head_state[(tb, h)] = (ops_, exsum)
            _, exsum = head_state[(tb, h)]
            sps = spsum.tile([128, TB], F32, tag="sp", name="sps")
            nc.tensor.matmul(
                sps[:, lo:TB],
                kT[hv][:, kt * 128 : (kt + 1) * 128],
                qts_t[(tb, h)][:, lo:TB],
                start=True,
                stop=True,
            )
            ex = expp.tile([128, TB], BF16, tag="ex", name="ex")
            nc.scalar.activation(ex[:, lo:TB], sps[:, lo:TB], EXP, scale=SCALE)
            if s["m"] >= 0:
                dcs = slice(128 * s["m"], 128 * (s["m"] + 1))
                nc.vector.tensor_mul(ex[:, dcs], ex[:, dcs], tri[:])
            if s["first"]:
                nc.vector.tensor_copy(exsum[:], ex[:])
            else:
                nc.vector.tensor_add(exsum[:, lo:TB], ex[:, lo:TB], exsum[:, lo:TB])
            s["ex"] = ex

        def emit_out_phase(s):
            tb, h, kt, lo = s["tb"], s["h"], s["kt"], s["lo"]
            hv = h // 2
            ops_, exsum = head_state[(tb, h)]
            nc.tensor.matmul(
                ops_[:, lo:TB],
                vT[kt][:, hv * 128 : (hv + 1) * 128],
                s["ex"][:, lo:TB],
                start=s["first"],
                stop=s["last"],
            )
            if s["last"]:
                # softmax denominator: reduce over k partitions AND broadcast
                # to all 128 partitions in one PE matmul against a ones matrix
                den = spsum.tile([128, TB], F32, tag="sp", name="den")
                nc.tensor.matmul(den[:], ones128[:], exsum[:], start=True, stop=True)
                sums = sump.tile([128, TB], F32, tag="sums", name="sums")
                nc.vector.reciprocal_approx_fast(sums[:], den[:])
                ot = outp.tile([HD, TB], BF16, tag="ot", name=f"ot{tb}_{h}")
                nc.vector.tensor_mul(ot[:], ops_[:], sums[:])
                outs_t[(tb, h)] = ot

        def run_segment(steps, fillers):
            fc = 0

            def emit_filler():
                nonlocal fc
                fn, tag = fillers[fc]
                fn()
                if tag is not None:
                    emitted.add(tag)
                fc += 1

            pend = deque()
            for s in steps:
                while fc < len(fillers) and not all(t in emitted for t in s["needs"]):
                    emit_filler()
                emit_score_phase(s)
                pend.append(s)
                if len(pend) > LOOKAHEAD:
                    emit_out_phase(pend.popleft())
                if fc < len(fillers):
                    emit_filler()
            while pend:
                emit_out_phase(pend.popleft())
            while fc < len(fillers):
                emit_filler()

        for tb in range(NTB):
            run_segment(build_steps(tb), build_fillers(tb))
        run_segment([], build_wo_fillers(NTB - 1))

    nc.compile()
    return nc


def _host_consts():
    inv_freq = 1.0 / (10000.0 ** (np.arange(0, HD, 2, dtype=np.float32) / HD))
    t = np.arange(T, dtype=np.float32)
    freqs = np.outer(t, inv_freq)  # [T, HD/2]
    freqs = np.repeat(freqs, 2, axis=-1)  # [T, HD]
    cos = np.cos(freqs).astype(np.float32).T.copy()  # [HD, T]
    sin = np.sin(freqs).astype(np.float32).T.copy()
    # rotated-by-64 signed sin table: row d holds the multiplier that pairs
    # with x[(d+64)%128]; rows 64..127 carry -sin[0:64], rows 0..63 +sin[64:128]
    nsin = np.empty_like(sin)
    nsin[0:64, :] = sin[64:128, :]
    nsin[64:128, :] = -sin[0:64, :]

    bf = ml_dtypes.bfloat16
    kp = np.arange(128)[:, None]
    qf = np.arange(128)[None, :]
    tri = (kp <= qf).astype(bf)

    return {
        "cosdt": np.ascontiguousarray(cos.astype(bf)),
        "nsindt": np.ascontiguousarray(nsin.astype(bf)),
        "tridt": tri,
        "onesdt": np.ones((128, 128), dtype=np.float16),
    }


_NC_CACHE = None


def _get_nc():
    global _NC_CACHE
    if _NC_CACHE is None:
        _NC_CACHE = build_nc()
    return _NC_CACHE


def kernel(x, Wq, Wk, Wv, Wo, _trace=False):
    x = np.asarray(x, dtype=np.float32)
    Wq = np.asarray(Wq, dtype=np.float32)
    Wk = np.asarray(Wk, dtype=np.float32)
    Wv = np.asarray(Wv, dtype=np.float32)
    Wo = np.asarray(Wo, dtype=np.float32)

    nc = _get_nc()
    consts = _host_consts()

    bf = ml_dtypes.bfloat16
    # packed partition-major host layouts (see dram tensor comments)
    xbs = []
    for b in range(B):
        xT_b = x[b].T.astype(bf)  # [C, T]
        xbs.append(np.ascontiguousarray(
            xT_b.reshape(NCT, 128, NTB, TB).transpose(1, 2, 0, 3).reshape(128, NTB * NCT * TB)
        ))
    wqs, wks, wvs, wos = [], [], [], []
    for g in range(2):
        wqT_g = Wq[1024 * g : 1024 * (g + 1), :].T.astype(bf)  # [C, 1024]
        wqs.append(np.ascontiguousarray(
            wqT_g.reshape(NCT, 128, NQH, 128).transpose(1, 2, 0, 3).reshape(128, NQH * NCT * 128)
        ))
        wkT_g = Wk[512 * g : 512 * (g + 1), :].T.astype(bf)  # [C, 512]
        wks.append(np.ascontiguousarray(
            wkT_g.reshape(NCT, 128, NKV * HD).transpose(1, 0, 2).reshape(128, NCT * NKV * HD)
        ))
        wvT_g = Wv[512 * g : 512 * (g + 1), :].T.astype(bf)
        wvs.append(np.ascontiguousarray(
            wvT_g.reshape(NCT, 128, NKV * HD).transpose(1, 0, 2).reshape(128, NCT * NKV * HD)
        ))
        woT_g = Wo[:, 1024 * g : 1024 * (g + 1)].T.astype(bf)  # [1024, C]
        wos.append(np.ascontiguousarray(
            woT_g.reshape(NQH, 128, C).transpose(1, 0, 2).reshape(128, NQH * C)
        ))

    in_maps = []
    for c in range(8):
        b, g = c // 2, c % 2
        im = {
            "xb_d": xbs[b],
            "wq_d": wqs[g],
            "wk_d": wks[g],
            "wv_d": wvs[g],
            "wo_d": wos[g],
        }
        im.update(consts)
        in_maps.append(im)

    res = run_bass_kernel_spmd(nc, in_maps, core_ids=list(range(8)), trace=_trace)

    y = np.empty((B, T, C), dtype=np.float32)
    for b in range(B):
        ya = np.asarray(res.results[2 * b]["yT"]).astype(np.float32)
        yb = np.asarray(res.results[2 * b + 1]["yT"]).astype(np.float32)
        y[b] = (ya + yb).T
    if _trace:
        return y, res
    return y
